# revision 3
# baseline (speedup 1.0000x reference)
"""Trainium2 Bass kernel for nn_DPAtt2Attention.

Algorithm (validated vs reference at rel err ~7e-8 in fp32 numpy):
  Per sequence b:
    S = x Wq^T + bq ; S2 = S*S ; L = S2 S2^T
    C_sub = C[ids,:][:,ids] ; A = C_sub + 1e-4 I = alpha*I + P,
      P = C_sub - I is EXACTLY rank-16 (C = 0.1 F F^T + I), so use
      Nystrom-Woodbury with the first 32 positions J of the sequence:
        W  = P[:, J]           (gathered: 32 columns of C_sub)
        Mp = P[J, J],  K = W^T W,  N = pinv(Mp)   (Newton-Schulz)
        B  = alpha I + N K,    M2 = B^T B,  M2i = inv(M2)  (Newton-Schulz)
        Z  = M2i B^T N W^T     =>  A^{-1} = (1/alpha)(I - W Z)
    T = L A^{-1} = (L - (L W) Z)/alpha
    subdet/denom/softmax/ctx/out-proj/residual layernorm as in reference.

  Gather of W^T (= C_sub[J-rows,:]) is done ON DEVICE from the full
  (replicated) C_kernel in HBM with two chained dma_gathers:
    stage 1: row-gather of the 4x32 J-rows of a 4-sequence group,
      int16 view + transpose=True -> SBUF [128, 157, 128] (u16 element e
      of each row lands at partition e%128, rank e//128)
    stage 2: SBUF-source token gather: token 2*col+h reads the 128-row
      vector of u16-half h of fp32 column col; transpose=True restores
      fp32 pairs contiguously -> W^T for all 4 seqs stacked on partitions.

Sharding: pure data-parallel, 32 sequences per core, weights + C_kernel
replicated. kernel() is self-contained: takes FULL inputs, returns FULL
output.
"""

import os
import sys

import numpy as np

for _p in ("/opt/trn_rl_repo",):
    if _p not in sys.path:
        sys.path.append(_p)

import concourse.bass as bass
import concourse.mybir as mybir
import concourse.tile as tile
from concourse import bacc
from concourse.bass import ds, ts

F32 = mybir.dt.float32
I16 = mybir.dt.int16

B, L, D = 256, 200, 64
ITEMS = 10000
CPAD = 10048                      # padded row length (fp32 elems), 40192B % 256 == 0
N_CORES = 8
SEQ_PER_CORE = B // N_CORES       # 32
GROUP = 4                         # sequences per gather group (4*32 = 128 rows)
N_GROUPS = SEQ_PER_CORE // GROUP  # 8
SJ = 32                           # Nystrom columns (J = first 32 positions)
ALPHA = 1.0001
NS1_IT = 24
NS2_IT = 24
HEAD = 2
SCALE = 1.0 / np.sqrt(D // HEAD)  # 1/sqrt(32)
LN_EPS = 1e-12
L0, L1 = 128, L - 128             # partition chunks 128 + 72

# stage-1 gather split: row halves in int16 elems (each % 128 == 0)
SPLIT_A = 10240
SPLIT_B = 2 * CPAD - SPLIT_A      # 9856
RANKS_A = SPLIT_A // 128          # 80
RANKS_B = SPLIT_B // 128          # 77
RANKS = RANKS_A + RANKS_B         # 157


def _emit_seq(nc, tc, ctx, pools, consts, g, s, dram, group_tiles):
    """Emit instructions for one sequence (index b = g*GROUP + s on this core)."""
    b = g * GROUP + s
    (psum, seq, small) = pools
    C = consts
    Wt0, Wt1, Z4, negLWT4 = group_tiles
    xT_d, xr_d, amask_d, y_d = dram["xT"], dram["xr"], dram["amask"], dram["y"]

    chunks = ((0, L0), (128, L1))

    # ---- input DMAs ----
    xT = seq.tile([64, L], F32, tag="xT")
    nc.sync.dma_start(xT[:, :], xT_d[b])
    x0 = seq.tile([L0, D], F32, tag="x0")
    nc.sync.dma_start(x0[:, :], xr_d[b, 0:L0])
    x1 = seq.tile([L1, D], F32, tag="x1")
    nc.sync.dma_start(x1[:, :], xr_d[b, L0:L])
    mk0 = seq.tile([L0, L], F32, tag="mk0")
    nc.sync.dma_start(mk0[:, :], amask_d[b, 0:L0])
    mk1 = seq.tile([L1, L], F32, tag="mk1")
    nc.sync.dma_start(mk1[:, :], amask_d[b, L0:L])

    # ---- S^T = Wq x^T + bq ; S2T = S^T * S^T ----
    pST = psum.tile([64, L], F32, tag="ps", bufs=4)
    nc.tensor.matmul(pST[:, :], C["wq_t"][:, :], xT[:, :])
    ST = seq.tile([64, L], F32, tag="ST")
    nc.vector.tensor_scalar_add(ST[:, :], pST[:, :], C["bq"][:, 0:1])
    S2T = seq.tile([64, L], F32, tag="S2T")
    nc.vector.tensor_mul(S2T[:, :], ST[:, :], ST[:, :])

    # ---- S2 chunks (transpose of S2T) ----
    S2c = []
    for ci, (i0, P) in enumerate(chunks):
        pt = psum.tile([P, 64], F32, tag="ps", bufs=4)
        nc.tensor.transpose(pt[:, :], S2T[:, i0 : i0 + P], C["ident"][0:64, 0:64])
        t = seq.tile([P, 64], F32, tag=f"S2_{ci}")
        nc.vector.tensor_copy(t[:, :], pt[:, :])
        S2c.append(t)

    # ---- inner = S2^T W  [64, SJ] ----
    pin = psum.tile([64, SJ], F32, tag="ps", bufs=4)
    nc.tensor.matmul(pin[:, :], S2c[0][:, :], Wt0[:, ts(s, SJ)], start=True, stop=False)
    nc.tensor.matmul(
        pin[:, :], S2c[1][0:L1, :], Wt1[0:L1, ts(s, SJ)], start=False, stop=True
    )
    inner = seq.tile([64, SJ], F32, tag="inner")
    nc.vector.tensor_copy(inner[:, :], pin[:, :])

    # ---- LW^T into partitions [32s:32s+32] of negLWT4, negated ----
    pLWT = psum.tile([128, L], F32, tag="ps", bufs=4)
    nc.tensor.matmul(
        pLWT[ts(s, SJ), :],
        inner[:, :],
        S2T[:, :],
        tile_position=(0, s * SJ),
        skip_group_check=True,
    )
    nc.vector.tensor_scalar_mul(negLWT4[ts(s, SJ), :], pLWT[ts(s, SJ), :], -1.0)

    # ---- T chunks: (L - LW Z)/alpha ----
    Tc = []
    for ci, (i0, P) in enumerate(chunks):
        pT = psum.tile([P, L], F32, tag=f"pT_{ci}")
        nc.tensor.matmul(pT[:, :], S2T[:, i0 : i0 + P], S2T[:, :], start=True, stop=False)
        nc.tensor.matmul(
            pT[:, :],
            negLWT4[ts(s, SJ), i0 : i0 + P],
            Z4[ts(s, SJ), 0:L],
            start=False,
            stop=True,
            tile_position=(s * SJ, 0),
        )
        t = seq.tile([P, L], F32, tag=f"T_{ci}")
        nc.vector.tensor_scalar_mul(t[:, :], pT[:, :], 1.0 / ALPHA)
        Tc.append(t)

    # ---- T^T chunks ----
    TT = [
        seq.tile([L0, L], F32, tag="TT0", name="TT0"),
        seq.tile([L1, L], F32, tag="TT1", name="TT1"),
    ]
    # TT0[:, 0:128] = (T0[:, 0:128])^T ; TT0[:, 128:200] = (T1[:, 0:128])^T
    # TT1[:, 0:128] = (T0[:, 128:200])^T ; TT1[:, 128:200] = (T1[:, 128:200])^T
    p = psum.tile([L0, L0], F32, tag="ps", bufs=4)
    nc.tensor.transpose(p[:, :], Tc[0][:, 0:L0], C["ident"][:, :])
    nc.scalar.copy(TT[0][:, 0:L0], p[:, :])
    p = psum.tile([L0, L1], F32, tag="ps", bufs=4)
    nc.tensor.transpose(p[:, :], Tc[1][0:L1, 0:L0], C["ident"][0:L1, 0:L1])
    nc.scalar.copy(TT[0][:, L0:L], p[:, :])
    p = psum.tile([L1, L0], F32, tag="ps", bufs=4)
    nc.tensor.transpose(p[:, :], Tc[0][:, L0:L], C["ident"][:, :])
    nc.scalar.copy(TT[1][0:L1, 0:L0], p[:, :])
    p = psum.tile([L1, L1], F32, tag="ps", bufs=4)
    nc.tensor.transpose(p[:, :], Tc[1][0:L1, L0:L], C["ident"][0:L1, 0:L1])
    nc.scalar.copy(TT[1][0:L1, L0:L], p[:, :])

    # ---- diag (col + row), cross = T o T^T ----
    dcol = []
    crossm = []
    crossred = []
    md = []
    for ci, (i0, P) in enumerate(chunks):
        m = seq.tile([P, L], F32, tag=f"md_{ci}")
        dc = small.tile([P, 1], F32, tag=f"dcol_{ci}")
        nc.vector.scalar_tensor_tensor(
            m[:, :],
            Tc[ci][0:P, :],
            1.0,
            C[f"dmask{ci}"][0:P, :],
            mybir.AluOpType.mult,
            mybir.AluOpType.mult,
            accum_out=dc[:, 0:1],
        )
        md.append(m)
        dcol.append(dc)
        cm = seq.tile([P, L], F32, tag=f"crossm_{ci}")
        cr = small.tile([P, 1], F32, tag=f"crossred_{ci}")
        nc.vector.scalar_tensor_tensor(
            cm[:, :],
            Tc[ci][0:P, :],
            1.0,
            TT[ci][0:P, :],
            mybir.AluOpType.mult,
            mybir.AluOpType.mult,
            accum_out=cr[:, 0:1],
        )
        crossm.append(cm)
        crossred.append(cr)

    # d_row [1, L] via ones-matmul over the masked diag tiles
    pdr = psum.tile([1, L], F32, tag="ps", bufs=4)
    nc.tensor.matmul(pdr[:, :], C["ones_col"][:, 0:1], md[0][:, :], start=True, stop=False)
    nc.tensor.matmul(
        pdr[:, :], C["ones_col"][0:L1, 0:1], md[1][0:L1, :], start=False, stop=True
    )
    drow = small.tile([1, L], F32, tag="drow")
    nc.vector.tensor_copy(drow[:, :], pdr[:, :])

    # total cross sum [1,1]
    pcr = psum.tile([1, 1], F32, tag="ps", bufs=4)
    nc.tensor.matmul(
        pcr[:, :], C["ones_col"][:, 0:1], crossred[0][:, 0:1], start=True, stop=False
    )
    nc.tensor.matmul(
        pcr[:, :], C["ones_col"][0:L1, 0:1], crossred[1][0:L1, 0:1], start=False, stop=True
    )

    # scalars: denom = max(0.5*(sumd^2 - cross), 1e-9); q = -SCALE/denom
    sc = small.tile([1, 8], F32, tag="scal")
    nc.vector.reduce_sum(sc[:, 0:1], drow[:, :], axis=mybir.AxisListType.X)
    nc.vector.tensor_mul(sc[:, 1:2], sc[:, 0:1], sc[:, 0:1])          # sumd^2
    nc.vector.tensor_sub(sc[:, 2:3], sc[:, 1:2], pcr[:, 0:1])         # - cross
    nc.vector.tensor_scalar_mul(sc[:, 3:4], sc[:, 2:3], 0.5)
    nc.vector.tensor_scalar_max(sc[:, 4:5], sc[:, 3:4], 1e-9)
    nc.vector.reciprocal(sc[:, 5:6], sc[:, 4:5])
    nc.vector.tensor_scalar_mul(sc[:, 6:7], sc[:, 5:6], -float(SCALE))  # q

    # broadcast q to all partitions
    pq = psum.tile([128, 1], F32, tag="ps", bufs=4)
    nc.tensor.matmul(pq[:, :], C["ones_row"][0:1, :], sc[:, 6:7])
    qb = small.tile([128, 1], F32, tag="qb")
    nc.vector.tensor_copy(qb[:, 0:1], pq[:, :])

    # ---- score + softmax per chunk ----
    attn = []
    for ci, (i0, P) in enumerate(chunks):
        dsc = small.tile([P, 1], F32, tag=f"dsc_{ci}")
        nc.vector.tensor_scalar_mul(dsc[:, 0:1], dcol[ci][:, 0:1], -float(SCALE))
        mkd = seq.tile([P, L], F32, tag=f"mkd_{ci}")
        mk = (mk0, mk1)[ci]
        nc.vector.scalar_tensor_tensor(
            mkd[:, :],
            C[f"dmask{ci}"][0:P, :],
            dsc[:, 0:1],
            mk[:, :],
            mybir.AluOpType.mult,
            mybir.AluOpType.add,
        )
        # outer product chunk via K=1 matmul: d[i]*d[j]
        pout = psum.tile([P, L], F32, tag=f"pout_{ci}")
        nc.tensor.matmul(pout[:, :], drow[:, i0 : i0 + P], drow[:, :])
        sd = seq.tile([P, L], F32, tag=f"sd_{ci}")
        nc.vector.tensor_sub(sd[:, :], pout[:, :], crossm[ci][:, :])
        score = seq.tile([P, L], F32, tag=f"score_{ci}")
        nc.vector.scalar_tensor_tensor(
            score[:, :],
            sd[:, :],
            qb[0:P, 0:1],
            mkd[:, :],
            mybir.AluOpType.mult,
            mybir.AluOpType.add,
        )
        nmax = small.tile([P, 1], F32, tag=f"nmax_{ci}")
        nc.vector.reduce_max(nmax[:, 0:1], score[:, :], axis=mybir.AxisListType.X, negate=True)
        e = seq.tile([P, L], F32, tag=f"e_{ci}")
        rsum = small.tile([P, 1], F32, tag=f"rsum_{ci}")
        nc.scalar.activation(
            e[:, :],
            score[:, :],
            mybir.ActivationFunctionType.Exp,
            bias=nmax[:, 0:1],
            scale=1.0,
            accum_out=rsum[:, 0:1],
        )
        rinv = small.tile([P, 1], F32, tag=f"rinv_{ci}")
        nc.vector.reciprocal(rinv[:, 0:1], rsum[:, 0:1])
        a = seq.tile([P, L], F32, tag=f"attn_{ci}")
        nc.vector.tensor_scalar_mul(a[:, :], e[:, :], rinv[:, 0:1])
        attn.append(a)

    # ---- attn^T ----
    at = [
        seq.tile([L0, L], F32, tag="at0", name="at0"),
        seq.tile([L1, L], F32, tag="at1", name="at1"),
    ]
    p = psum.tile([L0, L0], F32, tag="ps", bufs=4)
    nc.tensor.transpose(p[:, :], attn[0][:, 0:L0], C["ident"][:, :])
    nc.scalar.copy(at[0][:, 0:L0], p[:, :])
    p = psum.tile([L0, L1], F32, tag="ps", bufs=4)
    nc.tensor.transpose(p[:, :], attn[1][0:L1, 0:L0], C["ident"][0:L1, 0:L1])
    nc.scalar.copy(at[0][:, L0:L], p[:, :])
    p = psum.tile([L1, L0], F32, tag="ps", bufs=4)
    nc.tensor.transpose(p[:, :], attn[0][:, L0:L], C["ident"][:, :])
    nc.scalar.copy(at[1][0:L1, 0:L0], p[:, :])
    p = psum.tile([L1, L1], F32, tag="ps", bufs=4)
    nc.tensor.transpose(p[:, :], attn[1][0:L1, L0:L], C["ident"][0:L1, 0:L1])
    nc.scalar.copy(at[1][0:L1, L0:L], p[:, :])

    # ---- V chunks ----
    Vc = []
    for ci, (i0, P) in enumerate(chunks):
        pV = psum.tile([P, D], F32, tag="ps", bufs=4)
        nc.tensor.matmul(pV[:, :], xT[:, i0 : i0 + P], C["wv_t"][:, :])
        v = seq.tile([P, D], F32, tag=f"V_{ci}")
        nc.vector.tensor_add(v[:, :], pV[:, :], C["bv_b"][0:P, :])
        Vc.append(v)

    # ---- ctx^T = V^T attn^T  [64, L] ----
    pcx = psum.tile([64, L], F32, tag="ps", bufs=4)
    nc.tensor.matmul(pcx[:, :], Vc[0][:, :], at[0][:, :], start=True, stop=False)
    nc.tensor.matmul(pcx[:, :], Vc[1][0:L1, :], at[1][0:L1, :], start=False, stop=True)
    ctxT = seq.tile([64, L], F32, tag="ctxT")
    nc.vector.tensor_copy(ctxT[:, :], pcx[:, :])

    # ---- out proj + residual + layernorm ----
    for ci, (i0, P) in enumerate(chunks):
        po = psum.tile([P, D], F32, tag="ps", bufs=4)
        nc.tensor.matmul(po[:, :], ctxT[:, i0 : i0 + P], C["wd_t"][:, :])
        xc_in = (x0, x1)[ci]
        o = seq.tile([P, D], F32, tag=f"o_{ci}")
        nc.vector.tensor_add(o[:, :], po[:, :], xc_in[:, :])
        nc.vector.tensor_add(o[:, :], o[:, :], C["bd_b"][0:P, :])
        musum = small.tile([P, 1], F32, tag=f"musum_{ci}")
        nc.vector.reduce_sum(musum[:, 0:1], o[:, :], axis=mybir.AxisListType.X)
        mu = small.tile([P, 1], F32, tag=f"mu_{ci}")
        nc.vector.tensor_scalar_mul(mu[:, 0:1], musum[:, 0:1], 1.0 / D)
        xc = seq.tile([P, D], F32, tag=f"xc_{ci}")
        vsum = small.tile([P, 1], F32, tag=f"vsum_{ci}")
        nc.vector.tensor_scalar_sub(xc[:, :], o[:, :], mu[:, 0:1])
        sq = seq.tile([P, D], F32, tag=f"sq_{ci}")
        nc.vector.scalar_tensor_tensor(
            sq[:, :],
            xc[:, :],
            1.0,
            xc[:, :],
            mybir.AluOpType.mult,
            mybir.AluOpType.mult,
            accum_out=vsum[:, 0:1],
        )
        sdv = small.tile([P, 1], F32, tag=f"sdv_{ci}")
        nc.scalar.activation(
            sdv[:, 0:1],
            vsum[:, 0:1],
            mybir.ActivationFunctionType.Sqrt,
            bias=C["eps_col"][0:P, 0:1],
            scale=1.0 / D,
        )
        rstd = small.tile([P, 1], F32, tag=f"rstd_{ci}")
        nc.vector.reciprocal(rstd[:, 0:1], sdv[:, 0:1])
        xn = seq.tile([P, D], F32, tag=f"xn_{ci}")
        nc.vector.tensor_scalar_mul(xn[:, :], xc[:, :], rstd[:, 0:1])
        yv = seq.tile([P, D], F32, tag=f"y_{ci}")
        nc.vector.tensor_mul(yv[:, :], xn[:, :], C["gam_b"][0:P, :])
        nc.vector.tensor_add(yv[:, :], yv[:, :], C["bet_b"][0:P, :])
        nc.sync.dma_start(y_d[b, i0 : i0 + P], yv[:, :])


def _emit_group(nc, tc, ctx, pools, consts, g, dram):
    """Gathers + Nystrom/Woodbury small-matrix math for one 4-sequence group."""
    (psum, grp, gath, small) = pools
    C = consts

    # ---- stage 1: row gather (int16, transpose), split by row halves to
    # stay under the 128-deep SWDGE descriptor FIFO ----
    out1 = gath.tile([128, RANKS, 128], I16, tag="out1")
    nc.gpsimd.dma_gather(
        out1[:, 0:RANKS_A, :],
        dram["cpad"][:, 0:SPLIT_A],
        C["idx1"][:, g, :],
        128,
        128,
        SPLIT_A,
        elem_step=2 * CPAD,
        transpose=True,
    )
    nc.gpsimd.dma_gather(
        out1[:, RANKS_A:RANKS, :],
        dram["cpad"][:, SPLIT_A : 2 * CPAD],
        C["idx1"][:, g, :],
        128,
        128,
        SPLIT_B,
        elem_step=2 * CPAD,
        transpose=True,
    )

    # ---- stage 2: SBUF-source token gather -> W^T stacked for 4 seqs ----
    # (split per sequence: a single 2048-token gather crashes the HW ring)
    out2 = gath.tile([128, 1, 2048], I16, tag="out2")
    for s in range(GROUP):
        nc.gpsimd.dma_gather(
            out2[:, :, 512 * s : 512 * s + 512],
            out1[:, :, :].rearrange("p a b -> p (a b)"),
            C["idx2"][:, g, 32 * s : 32 * s + 32],
            512,
            400,
            128,
            transpose=True,
            sbuf_tokens_per_rank=128,
            sbuf_free_dim_per_rank=256,
        )
    o2f = out2[:, 0, :].bitcast(F32)  # [128, 1024]

    W4T = grp.tile([128, 256], F32, tag="W4T")
    nc.vector.memset(W4T[:, 200:256], 0.0)
    for s in range(GROUP):
        nc.vector.tensor_copy(W4T[ts(s, SJ), 0:200], o2f[ts(s, SJ), 256 * s : 256 * s + 200])
    # subtract the identity part: W = C_sub[:, J] - I[:, J]
    nc.vector.tensor_sub(W4T[:, 0:SJ], W4T[:, 0:SJ], C["i32pad"][:, :])

    # ---- W (transposed) chunks ----
    pt = psum.tile([128, 128], F32, tag="ps", bufs=4)
    nc.tensor.transpose(pt[:, :], W4T[:, 0:128], C["ident"][:, :])
    Wt0 = grp.tile([128, 128], F32, tag="Wt0")
    nc.vector.tensor_copy(Wt0[:, :], pt[:, :])
    pt = psum.tile([128, 128], F32, tag="ps", bufs=4)
    nc.tensor.transpose(pt[:, :], W4T[:, 128:256], C["ident"][:, :])
    Wt1 = grp.tile([128, 128], F32, tag="Wt1")
    nc.vector.tensor_copy(Wt1[:, :], pt[:, :])

    # ---- Mp4 (block-diag) via jsel matmul ----
    pm = psum.tile([128, 128], F32, tag="ps", bufs=4)
    nc.tensor.matmul(pm[:, :], Wt0[:, :], C["jsel"][:, :])
    Mp4 = grp.tile([128, 128], F32, tag="Mp4")
    nc.vector.tensor_mul(Mp4[:, :], pm[:, :], C["blkones"][:, :])

    # ---- K4 = W^T W (block-diag masked) ----
    pk = psum.tile([128, 128], F32, tag="ps", bufs=4)
    nc.tensor.matmul(pk[:, :], Wt0[:, :], Wt0[:, :], start=True, stop=False)
    nc.tensor.matmul(pk[:, :], Wt1[0:L1, :], Wt1[0:L1, :], start=False, stop=True)
    K4 = grp.tile([128, 128], F32, tag="K4")
    nc.vector.tensor_mul(K4[:, :], pk[:, :], C["blkones"][:, :])

    def _block_trace_bcast(M, tag):
        """per-32-block trace of M, broadcast to its partitions: [128,1]."""
        scr = grp.tile([128, 128], F32, tag="trscr")
        dv = small.tile([128, 1], F32, tag=f"dv_{tag}")
        nc.vector.scalar_tensor_tensor(
            scr[:, :], M[:, :], 1.0, C["ident"][:, :],
            mybir.AluOpType.mult, mybir.AluOpType.mult, accum_out=dv[:, 0:1],
        )
        ptr = psum.tile([128, 1], F32, tag="ps", bufs=4)
        nc.tensor.matmul(ptr[:, :], C["blkones"][:, :], dv[:, 0:1])
        trs = small.tile([128, 1], F32, tag=f"trs_{tag}", name="trs")
        nc.vector.tensor_copy(trs[:, 0:1], ptr[:, :])
        return trs

    def _ns(Minp, n_iter, x0_from_m, tag):
        """Newton-Schulz: pinv (x0 = M/tr^2) or inv (x0 = I/tr)."""
        ptr = _block_trace_bcast(Minp, tag)
        c = small.tile([128, 1], F32, tag=f"c_{tag}")
        if x0_from_m:
            nc.vector.tensor_mul(c[:, 0:1], ptr[:, :], ptr[:, :])
            nc.vector.reciprocal(c[:, 0:1], c[:, 0:1])
            X = grp.tile([128, 128], F32, tag=f"X_{tag}")
            nc.vector.tensor_scalar_mul(X[:, :], Minp[:, :], c[:, 0:1])
        else:
            nc.vector.reciprocal(c[:, 0:1], ptr[:, :])
            X = grp.tile([128, 128], F32, tag=f"X_{tag}")
            nc.vector.tensor_scalar_mul(X[:, :], C["ident"][:, :], c[:, 0:1])
        for _ in range(n_iter):
            pY = psum.tile([128, 128], F32, tag="ps", bufs=4)
            nc.tensor.matmul(pY[:, :], Minp[:, :], X[:, :])
            R = grp.tile([128, 128], F32, tag=f"R_{tag}")
            nc.vector.tensor_sub(R[:, :], C["two_eye"][:, :], pY[:, :])
            pX = psum.tile([128, 128], F32, tag="ps", bufs=4)
            nc.tensor.matmul(pX[:, :], X[:, :], R[:, :])
            X = grp.tile([128, 128], F32, tag=f"X_{tag}")
            nc.vector.tensor_copy(X[:, :], pX[:, :])
        return X

    N4 = _ns(Mp4, NS1_IT, True, "ns1")

    # ---- B = alpha I + N K ; M2 = B^T B ; M2i = inv(M2) ----
    pB = psum.tile([128, 128], F32, tag="ps", bufs=4)
    nc.tensor.matmul(pB[:, :], N4[:, :], K4[:, :])
    B4 = grp.tile([128, 128], F32, tag="B4")
    nc.vector.tensor_add(B4[:, :], pB[:, :], C["alpha_eye"][:, :])
    pM2 = psum.tile([128, 128], F32, tag="ps", bufs=4)
    nc.tensor.matmul(pM2[:, :], B4[:, :], B4[:, :])
    M2 = grp.tile([128, 128], F32, tag="M2")
    nc.vector.tensor_copy(M2[:, :], pM2[:, :])
    M2i = _ns(M2, NS2_IT, False, "ns2")

    # ---- Z4 = M2i B^T N W^T  [128, 256] ----
    pY4 = psum.tile([128, 256], F32, tag="ps", bufs=4)
    nc.tensor.matmul(pY4[:, :], N4[:, :], W4T[:, :])
    Y4 = grp.tile([128, 256], F32, tag="Y4")
    nc.vector.tensor_copy(Y4[:, :], pY4[:, :])
    pP1 = psum.tile([128, 256], F32, tag="ps", bufs=4)
    nc.tensor.matmul(pP1[:, :], B4[:, :], Y4[:, :])
    P1 = grp.tile([128, 256], F32, tag="P1")
    nc.vector.tensor_copy(P1[:, :], pP1[:, :])
    pZ4 = psum.tile([128, 256], F32, tag="ps", bufs=4)
    nc.tensor.matmul(pZ4[:, :], M2i[:, :], P1[:, :])
    Z4 = grp.tile([128, 256], F32, tag="Z4")
    nc.vector.tensor_copy(Z4[:, :], pZ4[:, :])

    negLWT4 = grp.tile([128, L], F32, tag="negLWT4")
    return Wt0, Wt1, Z4, negLWT4


def build_program(n_groups=N_GROUPS, repeat=1):
    """Build the per-core Bass/Tile program. Returns (nc, input_names)."""
    nc = bacc.Bacc("TRN2", target_bir_lowering=False, debug=False)

    nseq = n_groups * GROUP
    dram = {
        "cpad": nc.dram_tensor("cpad", [ITEMS, 2 * CPAD], I16, kind="ExternalInput").ap(),
        "xT": nc.dram_tensor("xT", [nseq, 64, L], F32, kind="ExternalInput").ap(),
        "xr": nc.dram_tensor("xr", [nseq, L, D], F32, kind="ExternalInput").ap(),
        "amask": nc.dram_tensor("amask", [nseq, L, L], F32, kind="ExternalInput").ap(),
        "idx1": nc.dram_tensor("idx1", [n_groups, 128, 8], I16, kind="ExternalInput").ap(),
        "idx2": nc.dram_tensor("idx2", [n_groups, 128, 128], I16, kind="ExternalInput").ap(),
        "y": nc.dram_tensor("y", [nseq, L, D], F32, kind="ExternalOutput").ap(),
    }
    const_specs = {
        "wq_t": ([64, 64], F32),
        "wv_t": ([64, 64], F32),
        "wd_t": ([64, 64], F32),
        "bq": ([64, 1], F32),
        "bv": ([1, 64], F32),
        "bd": ([1, 64], F32),
        "gam": ([1, 64], F32),
        "bet": ([1, 64], F32),
        "ident": ([128, 128], F32),
        "two_eye": ([128, 128], F32),
        "alpha_eye": ([128, 128], F32),
        "i32pad": ([128, SJ], F32),
        "jsel": ([128, 128], F32),
        "blkones": ([128, 128], F32),
        "ones_col": ([128, 1], F32),
        "eps_col": ([128, 1], F32),
        "ones_row": ([1, 128], F32),
        "dmask0": ([128, L], F32),
        "dmask1": ([128, L], F32),
    }
    for name, (shape, dt) in const_specs.items():
        dram[name] = nc.dram_tensor(name, shape, dt, kind="ExternalInput").ap()

    with tile.TileContext(nc) as tc:
        import contextlib

        with contextlib.ExitStack() as ctx:
            cpool = ctx.enter_context(tc.tile_pool(name="consts", bufs=1))
            psum = ctx.enter_context(tc.tile_pool(name="psum", bufs=1, space="PSUM"))
            grp = ctx.enter_context(tc.tile_pool(name="grp", bufs=2))
            gath = ctx.enter_context(tc.tile_pool(name="gath", bufs=2))
            seq = ctx.enter_context(tc.tile_pool(name="seq", bufs=2))
            small = ctx.enter_context(tc.tile_pool(name="small", bufs=3))

            # load constants
            C = {}
            for name, (shape, dt) in const_specs.items():
                t = cpool.tile(shape, dt, tag=f"c_{name}")
                nc.sync.dma_start(t[...], dram[name][...])
                C[name] = t
            idx1 = cpool.tile([128, n_groups, 8], I16, tag="c_idx1")
            nc.sync.dma_start(idx1[:, :, :], dram["idx1"].rearrange("g p s -> p g s"))
            C["idx1"] = idx1
            idx2 = cpool.tile([128, n_groups, 128], I16, tag="c_idx2")
            nc.sync.dma_start(idx2[:, :, :], dram["idx2"].rearrange("g p s -> p g s"))
            C["idx2"] = idx2

            # partition-broadcast row constants
            for rname in ("bv", "bd", "gam", "bet"):
                bt = cpool.tile([128, 64], F32, tag=f"c_{rname}_b")
                nc.gpsimd.partition_broadcast(bt[:, :], C[rname][0:1, :])
                C[rname + "_b"] = bt

            def _emit_all():
                for g in range(n_groups):
                    gtiles = _emit_group(
                        nc, tc, ctx, (psum, grp, gath, small), C, g, dram
                    )
                    for s in range(GROUP):
                        _emit_seq(
                            nc, tc, ctx, (psum, seq, small), C, g, s, dram, gtiles
                        )

            if repeat == 1:
                _emit_all()
            else:
                with tc.For_i(0, repeat):
                    _emit_all()

    nc.compile()
    return nc, dram


# ----------------------------------------------------------------------------
# Host-side prep
# ----------------------------------------------------------------------------

def host_prep(inputs, n_groups=N_GROUPS):
    """Build per-core input maps from full inputs."""
    x = np.ascontiguousarray(np.asarray(inputs["input_tensor"], dtype=np.float32))
    amask = np.ascontiguousarray(np.asarray(inputs["attention_mask"], dtype=np.float32))
    ids = np.asarray(inputs["seq_item_ids"]).astype(np.int64)
    Ck = np.asarray(inputs["C_kernel"], dtype=np.float32)

    cpad = np.zeros((ITEMS, CPAD), dtype=np.float32)
    cpad[:, :ITEMS] = Ck
    cpad_i16 = cpad.view(np.int16).reshape(ITEMS, 2 * CPAD)

    def wrap16(vals, width):
        """layout [128, width]: item i at partition i%16 (replicated x8), free i//16."""
        a = np.asarray(vals, dtype=np.int16).reshape(width, 16).T  # [16, width]
        return np.tile(a, (8, 1))

    consts = {
        "wq_t": np.ascontiguousarray(np.asarray(inputs["Wq"], np.float32).T),
        "wv_t": np.ascontiguousarray(np.asarray(inputs["Wv"], np.float32).T),
        "wd_t": np.ascontiguousarray(np.asarray(inputs["Wd"], np.float32).T),
        "bq": np.asarray(inputs["bq"], np.float32).reshape(64, 1),
        "bv": np.asarray(inputs["bv"], np.float32).reshape(1, 64),
        "bd": np.asarray(inputs["bd"], np.float32).reshape(1, 64),
        "gam": np.asarray(inputs["ln_gamma"], np.float32).reshape(1, 64),
        "bet": np.asarray(inputs["ln_beta"], np.float32).reshape(1, 64),
        "ident": np.eye(128, dtype=np.float32),
        "two_eye": 2.0 * np.eye(128, dtype=np.float32),
        "alpha_eye": np.float32(ALPHA) * np.eye(128, dtype=np.float32),
        "i32pad": np.tile(np.eye(SJ, dtype=np.float32), (4, 1)),
        "jsel": (np.arange(128)[:, None] == (np.arange(128)[None, :] % SJ)).astype(
            np.float32
        ),
        "blkones": (np.arange(128)[:, None] // SJ == np.arange(128)[None, :] // SJ).astype(
            np.float32
        ),
        "ones_col": np.ones((128, 1), np.float32),
        "eps_col": np.full((128, 1), LN_EPS, np.float32),
        "ones_row": np.ones((1, 128), np.float32),
        "dmask0": (np.arange(128)[:, None] == np.arange(L)[None, :]).astype(np.float32),
        "dmask1": ((np.arange(128)[:, None] + 128) == np.arange(L)[None, :]).astype(
            np.float32
        ),
    }

    nseq = n_groups * GROUP
    in_maps = []
    for k in range(N_CORES):
        sl = slice(k * SEQ_PER_CORE, k * SEQ_PER_CORE + nseq)
        xk = x[sl]
        idk = ids[sl]
        idx1 = np.zeros((n_groups, 128, 8), np.int16)
        idx2 = np.zeros((n_groups, 128, 128), np.int16)
        for g in range(n_groups):
            jrows = idk[g * GROUP : (g + 1) * GROUP, :SJ].reshape(-1)  # 128 row ids
            idx1[g] = wrap16(jrows, 8)
            toks = np.full((GROUP, 512), -1, np.int64)
            for s in range(GROUP):
                e = np.empty(400, np.int64)
                e[0::2] = 2 * idk[g * GROUP + s]
                e[1::2] = 2 * idk[g * GROUP + s] + 1
                # map u16-element index -> stage-1 token (rank-window split)
                t = np.where(e < SPLIT_A, e, RANKS_A * 128 + (e - SPLIT_A))
                toks[s, :400] = t
            idx2[g] = wrap16(toks.reshape(-1), 128)
        m = {
            "cpad": cpad_i16,
            "xT": np.ascontiguousarray(xk.transpose(0, 2, 1)),
            "xr": xk,
            "amask": amask[sl],
            "idx1": idx1,
            "idx2": idx2,
        }
        m.update(consts)
        in_maps.append(m)
    return in_maps


_PROGRAM_CACHE = {}


def kernel(**inputs) -> np.ndarray:
    from concourse.bass_utils import run_bass_kernel_spmd

    if "nc" not in _PROGRAM_CACHE:
        _PROGRAM_CACHE["nc"] = build_program()
    nc, _ = _PROGRAM_CACHE["nc"]
    in_maps = host_prep(inputs)
    res = run_bass_kernel_spmd(nc, in_maps, list(range(N_CORES)))
    outs = [res.results[k]["y"] for k in range(N_CORES)]
    return np.concatenate(outs, axis=0).astype(np.float32)

